# revision 10
# baseline (speedup 1.0000x reference)
"""Trainium2 Bass kernel for CLVP conditioning-encoder self-attention.

Problem: B=2, S=2048, E=1024, 16 heads x 64 dim, T5-style relative position
bias (32 buckets, bidirectional=False), softmax, output projection.

Sharding: 8 cores = 2 batches x 4 head-groups (4 heads each). Each core
computes, for its (batch, 4 heads): QKV projections, attention, and the
partial output projection ctx_part @ Wo[head_slice]. The host sums the 4
partials per batch (row-parallel Megatron gather) and adds bo.

Device layout (everything transposed => no on-chip transposes needed):
  hsT  [E=1024, S=2048]  hidden states, transposed on host
  qT/kT = Wq/Wk^T-slices applied via matmul(lhsT=W tiles, rhs=hsT) -> [256, S]
  v    [S, 260] = hs @ Wv_aug  (4 heads x (64 dims + ones column))
  scoresT[k_tok, q] per (head, k-tile) with PSUM accumulation
  relative bias: bias(q,k) = vec_h[k - q + 2047]; for k > q it is the
  constant c_h = 8*rel_bias[bucket0, h]. The varying (k <= q) triangle is
  added by DVE from a host-built Toeplitz master M'_h[p, j] =
  vec_h[p - j + 2047] - c_h (zero in its own upper triangle), and c_h is
  applied for free as the ACT exp() per-partition bias.
  attn@V: lhsT = V_aug[k,65], rhs = expT -> ctx_augT [65, q] in PSUM; row 64
  is the softmax denominator (ones column). Normalize with
  reciprocal_approx_fast + gpsimd partition_broadcast + DVE multiply.
  O-proj: outT[e, s] = Wo_slice^T-tiles @ ctxT -> [1024, 2048] partial.
"""

import numpy as np

import concourse.bass as bass
import concourse.tile as tile
from concourse import bacc, mybir
from concourse import bass_utils

F32 = mybir.dt.float32
F32R = mybir.dt.float32r
EXP = mybir.ActivationFunctionType.Exp

E = 1024
S = 2048
NH = 16
HC = 4            # heads per core
HD = 64
ECORE = HC * HD   # 256
VW = HC * (HD + 1)  # 260, v with ones columns
NCORES = 8
NB = 32
MAXD = 64
SCALE = 1.0 / np.sqrt(HD)


def _rel_bucket(d):
    """numpy mirror of reference._relative_position_bucket (f32 semantics)."""
    rp = np.maximum(-d, 0)
    max_exact = NB // 2
    is_small = rp < max_exact
    rp_f = np.maximum(rp, 1).astype(np.float32)
    large = max_exact + (
        np.log(rp_f / np.float32(max_exact))
        / np.float32(np.log(MAXD / max_exact))
        * np.float32(NB - max_exact)
    ).astype(np.int32)
    large = np.minimum(large, NB - 1)
    return np.where(is_small, rp, large)


def _emit_body(nc, pool, psum, t):
    """Emit one full forward pass. t: dict of DRAM APs."""
    mm = nc.tensor.matmul

    def rr(ap):
        return ap

    # ---- constants / weights to SBUF ----
    wq_sb, wk_sb, wv_sb = [], [], []
    for kk in range(8):
        a = pool.tile([128, ECORE], F32R, tag="w", bufs=24, name=f"wq{kk}")
        nc.sync.dma_start(out=a, in_=t["wq"][kk * 128:(kk + 1) * 128, :])
        wq_sb.append(a)
        b = pool.tile([128, ECORE], F32R, tag="w", bufs=24, name=f"wk{kk}")
        nc.sync.dma_start(out=b, in_=t["wk"][kk * 128:(kk + 1) * 128, :])
        wk_sb.append(b)
        c = pool.tile([128, VW], F32R, tag="w", bufs=24, name=f"wv{kk}")
        nc.sync.dma_start(out=c, in_=t["wv"][kk * 128:(kk + 1) * 128, :])
        wv_sb.append(c)
    bq_sb = pool.tile([1, ECORE], F32R, tag="brow", bufs=3, name="bqs")
    nc.sync.dma_start(out=bq_sb, in_=t["bq"][:, :])
    bk_sb = pool.tile([1, ECORE], F32R, tag="brow", bufs=3, name="bks")
    nc.sync.dma_start(out=bk_sb, in_=t["bk"][:, :])
    bv_sb = pool.tile([1, VW], F32R, tag="brow", bufs=3, name="bvs")
    nc.sync.dma_start(out=bv_sb, in_=t["bv"][:, :])
    cvec_sb = pool.tile([128, HC], F32, tag="cvec", bufs=1, name="cvecs")
    nc.sync.dma_start(out=cvec_sb, in_=t["cvec"][:, :])
    ones_sb = pool.tile([1, 512], F32R, tag="ones", bufs=1, name="oness")
    nc.sync.dma_start(out=ones_sb, in_=t["ones"][:, :])

    # ---- hidden states (transposed) ----
    ht = []
    for kk in range(8):
        a = pool.tile([128, S], F32R, tag="b8", bufs=13, name=f"ht{kk}")
        nc.sync.dma_start(out=a, in_=t["hsT"][kk * 128:(kk + 1) * 128, :])
        ht.append(a)

    # ---- qT / kT projections: out [256, S] as 2 tiles of [128, S] ----
    qt_sb = [pool.tile([128, S], F32R, tag="b8", bufs=13, name=f"qt{mt}") for mt in range(2)]
    kt_sb = [pool.tile([128, S], F32R, tag="b8", bufs=13, name=f"kt{mt}") for mt in range(2)]
    for dst, w_sb, b_sb in ((qt_sb, wq_sb, bq_sb), (kt_sb, wk_sb, bk_sb)):
        for mt in range(2):
            ms = slice(mt * 128, (mt + 1) * 128)
            for nt in range(4):
                ns = slice(nt * 512, (nt + 1) * 512)
                ps = psum.tile([128, 512], F32, tag="ctx", name=f"pj{mt}{nt}")
                for kk in range(8):
                    mm(ps, lhsT=rr(w_sb[kk][:, ms]), rhs=rr(ht[kk][:, ns]),
                       start=(kk == 0), stop=False)
                mm(ps, lhsT=rr(b_sb[0:1, ms]), rhs=rr(ones_sb[0:1, :]),
                   start=False, stop=True)
                nc.vector.tensor_copy(dst[mt][:, ns], ps)

    # ---- v projection: v[s, 260] per 128-token tile ----
    v_sb = []
    for st in range(16):
        ss = slice(st * 128, (st + 1) * 128)
        ps = psum.tile([128, VW], F32, tag="ctx", name=f"pv{st}")
        for kk in range(8):
            mm(ps, lhsT=rr(ht[kk][:, ss]), rhs=rr(wv_sb[kk]),
               start=(kk == 0), stop=False)
        mm(ps, lhsT=rr(ones_sb[0:1, 0:128]), rhs=rr(bv_sb[0:1, :]),
           start=False, stop=True)
        a = pool.tile([128, VW], F32R, tag="v", bufs=16, name=f"v{st}")
        nc.vector.tensor_copy(a, ps)
        v_sb.append(a)

    # ---- attention ----
    ctxT_sb = [pool.tile([128, S], F32R, tag="b8", bufs=13, name=f"cx{p}") for p in range(2)]
    for h in range(HC):
        pi, row0 = h // 2, 64 * (h % 2)
        mast_h = pool.tile([128, S], F32, tag="b8", bufs=13, name=f"mast{h}")
        nc.sync.dma_start(out=mast_h, in_=t["mast"][:, h, :])
        for qh in range(2):
            qo = qh * 1024
            ctx_ps = psum.tile([65, 1024], F32, tag="ctx", name=f"pc{h}{qh}")
            for kt in range(16):
                k0 = kt * 128
                s_ps = psum.tile([128, 1024], F32, tag="sc", name=f"sc{h}{qh}{kt}")
                for c in range(2):
                    cs = slice(c * 512, (c + 1) * 512)
                    mm(s_ps[:, cs],
                       lhsT=rr(kt_sb[pi][row0:row0 + 64, k0:k0 + 128]),
                       rhs=rr(qt_sb[pi][row0:row0 + 64, qo + c * 512: qo + (c + 1) * 512]),
                       start=True, stop=True)
                if k0 < qo + 1024:
                    v0 = max(0, k0 - qo)
                    j0 = qo + v0 - k0
                    w = 1024 - v0
                    nc.vector.tensor_add(s_ps[:, v0:1024], s_ps[:, v0:1024],
                                         mast_h[:, j0:j0 + w])
                e_sb = pool.tile([128, 1024], F32R, tag="exp", bufs=3, name=f"e{h}{qh}{kt}")
                nc.scalar.activation(e_sb, s_ps, EXP,
                                     bias=cvec_sb[:, h:h + 1], scale=1.0)
                for c in range(2):
                    cs = slice(c * 512, (c + 1) * 512)
                    mm(ctx_ps[:, cs],
                       lhsT=rr(v_sb[kt][:, h * 65:(h + 1) * 65]),
                       rhs=rr(e_sb[:, cs]),
                       start=(kt == 0), stop=(kt == 15))
            # exact reciprocal; reshape [1,1024]->[128,8] so 128 DVE lanes share
            # the iterative divide (DMA cannot read PSUM, so DVE-copy out first)
            denrow = pool.tile([1, 1024], F32, tag="recip", bufs=2, name=f"dr{h}{qh}")
            nc.vector.tensor_copy(denrow, ctx_ps[64:65, :])
            den = pool.tile([128, 8], F32, tag="den", bufs=4, name=f"d{h}{qh}")
            nc.sync.dma_start(out=den, in_=denrow)
            rec8 = pool.tile([128, 8], F32, tag="den", bufs=4, name=f"re{h}{qh}")
            nc.vector.reciprocal(rec8, den)
            recip = pool.tile([1, 1024], F32, tag="recip", bufs=2, name=f"r{h}{qh}")
            nc.sync.dma_start(out=recip, in_=rec8)
            rb = pool.tile([64, 1024], F32, tag="rb", bufs=2, name=f"rb{h}{qh}")
            nc.gpsimd.partition_broadcast(rb, recip[0:1, :])
            nc.vector.tensor_mul(ctxT_sb[pi][row0:row0 + 64, qo:qo + 1024],
                                 ctx_ps[0:64, :], rb)

    # ---- output projection: outT[e, s] partial ----
    wo_sb = []
    for ct in range(2):
        a = pool.tile([128, E], F32R, tag="w4k", bufs=2, name=f"wo{ct}")
        nc.sync.dma_start(out=a, in_=t["wo"][ct * 128:(ct + 1) * 128, :])
        wo_sb.append(a)
    for mt in range(8):
        ms = slice(mt * 128, (mt + 1) * 128)
        ot = pool.tile([128, S], F32, tag="b8", bufs=13, name=f"ot{mt}")
        for nt in range(4):
            ns = slice(nt * 512, (nt + 1) * 512)
            ps = psum.tile([128, 512], F32, tag="ctx", name=f"po{mt}{nt}")
            for ct in range(2):
                mm(ps, lhsT=rr(wo_sb[ct][:, ms]), rhs=rr(ctxT_sb[ct][:, ns]),
                   start=(ct == 0), stop=(ct == 1))
            nc.vector.tensor_copy(ot[:, ns], ps)
        nc.sync.dma_start(out=t["outT"][ms, :], in_=ot)


def build(iters=1, enable_asserts=False):
    """Build + compile the Bass program. Returns nc."""
    nc = bacc.Bacc("TRN2", target_bir_lowering=False, debug=False,
                   enable_asserts=enable_asserts, num_devices=NCORES)
    t = {
        "hsT": nc.dram_tensor("hsT", [E, S], F32R, kind="ExternalInput").ap(),
        "wq": nc.dram_tensor("wq", [E, ECORE], F32R, kind="ExternalInput").ap(),
        "wk": nc.dram_tensor("wk", [E, ECORE], F32R, kind="ExternalInput").ap(),
        "wv": nc.dram_tensor("wv", [E, VW], F32R, kind="ExternalInput").ap(),
        "bq": nc.dram_tensor("bq", [1, ECORE], F32R, kind="ExternalInput").ap(),
        "bk": nc.dram_tensor("bk", [1, ECORE], F32R, kind="ExternalInput").ap(),
        "bv": nc.dram_tensor("bv", [1, VW], F32R, kind="ExternalInput").ap(),
        "wo": nc.dram_tensor("wo", [ECORE, E], F32R, kind="ExternalInput").ap(),
        "ones": nc.dram_tensor("ones", [1, 512], F32R, kind="ExternalInput").ap(),
        "mast": nc.dram_tensor("mast", [128, HC, S], F32, kind="ExternalInput").ap(),
        "cvec": nc.dram_tensor("cvec", [128, HC], F32, kind="ExternalInput").ap(),
        "outT": nc.dram_tensor("outT", [E, S], F32, kind="ExternalOutput").ap(),
    }
    with tile.TileContext(nc) as tc:
        with tc.tile_pool(name="sb", bufs=1) as pool, \
             tc.tile_pool(name="ps", bufs=2, space="PSUM") as psum:
            # tag capacity: b8 tiles are the big [128, 2048] f32 workhorses
            for _ in range(iters):
                _emit_body(nc, pool, psum, t)
    nc.compile()
    return nc


def make_in_maps(inputs):
    """Shard full inputs into 8 per-core input maps."""
    hs = np.asarray(inputs["hidden_states"], dtype=np.float32)
    Wq = np.asarray(inputs["Wq"], dtype=np.float32)
    bq = np.asarray(inputs["bq"], dtype=np.float32)
    Wk = np.asarray(inputs["Wk"], dtype=np.float32)
    bk = np.asarray(inputs["bk"], dtype=np.float32)
    Wv = np.asarray(inputs["Wv"], dtype=np.float32)
    bv = np.asarray(inputs["bv"], dtype=np.float32)
    Wo = np.asarray(inputs["Wo"], dtype=np.float32)
    rel = np.asarray(inputs["rel_bias"], dtype=np.float32)

    # bias lookup vector per head: vec_h[x] = 8*rel[bucket(x-2047), h],
    # x in [0, 2174] used by the masters
    d = np.arange(-2047, 128)          # x - 2047 for x in [0, 2175)
    buck = _rel_bucket(d)              # [2175]
    idx = (np.arange(128)[:, None] - np.arange(S)[None, :]) + 2047  # [128, 2048]

    in_maps = []
    for core in range(NCORES):
        b, hg = core // 4, core % 4
        h0 = hg * HC
        sl = slice(h0 * HD, (h0 + HC) * HD)
        wv_aug = np.zeros((E, VW), np.float32)
        bv_aug = np.zeros((1, VW), np.float32)
        mast = np.zeros((128, HC, S), np.float32)
        cvec = np.zeros((128, HC), np.float32)
        for h in range(HC):
            wv_aug[:, h * 65:h * 65 + 64] = Wv[:, (h0 + h) * HD:(h0 + h + 1) * HD]
            bv_aug[0, h * 65:h * 65 + 64] = bv[(h0 + h) * HD:(h0 + h + 1) * HD]
            bv_aug[0, h * 65 + 64] = 1.0
            vec_h = 8.0 * rel[buck, h0 + h]          # [2175]
            c_h = 8.0 * rel[0, h0 + h]               # bucket-0 constant
            mast[:, h, :] = vec_h[idx] - c_h
            cvec[:, h] = c_h
        in_maps.append({
            "hsT": np.ascontiguousarray(hs[b].T),
            "wq": Wq[:, sl] * np.float32(SCALE),
            "wk": np.ascontiguousarray(Wk[:, sl]),
            "wv": wv_aug,
            "bq": (bq[sl] * np.float32(SCALE)).reshape(1, ECORE),
            "bk": bk[sl].reshape(1, ECORE),
            "bv": bv_aug,
            "wo": np.ascontiguousarray(Wo[sl, :]),
            "ones": np.ones((1, 512), np.float32),
            "mast": mast,
            "cvec": cvec,
        })
    return in_maps


def gather(results, inputs):
    """Sum per-core partial outT's into the full [B, S, E] output."""
    bo = np.asarray(inputs["bo"], dtype=np.float32)
    B = np.asarray(inputs["hidden_states"]).shape[0]
    out = np.zeros((B, S, E), np.float32)
    for b in range(B):
        acc = np.zeros((E, S), np.float32)
        for hg in range(4):
            acc += results[b * 4 + hg]["outT"]
        out[b] = acc.T + bo
    return out


_NC = None


def kernel(**inputs):
    global _NC
    if _NC is None:
        _NC = build()
    in_maps = make_in_maps(inputs)
    res = bass_utils.run_bass_kernel_spmd(_NC, in_maps, core_ids=list(range(NCORES)))
    return gather(res.results, inputs)


# revision 28
# speedup vs baseline: 369.0895x; 369.0895x over previous
"""Trainium2 Bass kernel for CLVP conditioning-encoder self-attention.

Problem: B=2, S=2048, E=1024, 16 heads x 64 dim, T5-style relative position
bias (32 buckets, bidirectional=False), softmax, output projection.

Sharding: 8 cores = 2 batches x 4 head-groups (4 heads each). Each core
computes, for its (batch, 4 heads): QKV projections, attention, and the
partial output projection ctx_part @ Wo[head_slice]. The host sums the 4
partials per batch (row-parallel Megatron gather) and adds bo.

Device layout (everything transposed => no on-chip transposes needed):
  hsT  [E=1024, S=2048]  hidden states, transposed on host
  qT/kT = Wq/Wk^T-slices applied via matmul(lhsT=W tiles, rhs=hsT) -> [256, S]
  v    [S, 260] = hs @ Wv_aug  (4 heads x (64 dims + ones column))
  scoresT[k_tok, q] per (head, k-tile) with PSUM accumulation
  relative bias: bias(q,k) = vec_h[k - q + 2047]; for k > q it is the
  constant c_h = 8*rel_bias[bucket0, h]. The varying (k <= q) triangle is
  added by DVE from a host-built Toeplitz master M'_h[p, j] =
  vec_h[p - j + 2047] - c_h (zero in its own upper triangle), and c_h is
  applied for free as the ACT exp() per-partition bias.
  attn@V: lhsT = V_aug[k,65], rhs = expT -> ctx_augT [65, q] in PSUM; row 64
  is the softmax denominator (ones column). Normalize with
  reciprocal_approx_fast + gpsimd partition_broadcast + DVE multiply.
  O-proj: outT[e, s] = Wo_slice^T-tiles @ ctxT -> [1024, 2048] partial.
"""

import numpy as np

import concourse.tile as tile
from concourse import bacc, mybir
from concourse import bass_utils

F32 = mybir.dt.float32
F32R = mybir.dt.float32r
EXP = mybir.ActivationFunctionType.Exp

E = 1024
S = 2048
NH = 16
HC = 4            # heads per core
HD = 64
ECORE = HC * HD   # 256
VW = HC * (HD + 1)  # 260, v with ones columns
NCORES = 8
NB = 32
MAXD = 64
SCALE = 1.0 / np.sqrt(HD)
# bucket saturates at relative distance >= RP_SAT -> bias constant (v31) there.
# Within a 128-row k-tile the varying "band" spans q in [k0, k0 + BW).
RP_SAT = 59
BW = 127 + RP_SAT          # 186
BWPAD = 192


def _rel_bucket(d):
    """numpy mirror of reference._relative_position_bucket (f32 semantics)."""
    rp = np.maximum(-d, 0)
    max_exact = NB // 2
    is_small = rp < max_exact
    rp_f = np.maximum(rp, 1).astype(np.float32)
    large = max_exact + (
        np.log(rp_f / np.float32(max_exact))
        / np.float32(np.log(MAXD / max_exact))
        * np.float32(NB - max_exact)
    ).astype(np.int32)
    large = np.minimum(large, NB - 1)
    return np.where(is_small, rp, large)


def _emit_body(nc, pool, psum, t, opts):
    """Emit one full forward pass. t: dict of DRAM APs."""
    mm = nc.tensor.matmul
    sc_bufs = opts.get("sc_bufs", 5)
    ctx_bufs = opts.get("ctx_bufs", 3)
    exp_bufs = opts.get("exp_bufs", 8)
    paired = opts.get("paired", True)
    evac = nc.scalar.copy if opts.get("act_evac", True) else nc.vector.tensor_copy

    def rr(ap):
        return ap

    # ---- hidden states first (largest DMA, on the projection critical path)
    ht = []
    for kk in range(8):
        a = pool.tile([128, S], F32R, tag="b8", bufs=13, name=f"ht{kk}")
        nc.sync.dma_start(out=a, in_=t["hsT"][kk * 128:(kk + 1) * 128, :])
        ht.append(a)

    # ---- constants / weights to SBUF ----
    wq_sb, wk_sb, wv_sb = [], [], []
    for kk in range(8):
        a = pool.tile([128, ECORE], F32R, tag="w", bufs=24, name=f"wq{kk}")
        nc.sync.dma_start(out=a, in_=t["wq"][kk * 128:(kk + 1) * 128, :])
        wq_sb.append(a)
        b = pool.tile([128, ECORE], F32R, tag="w", bufs=24, name=f"wk{kk}")
        nc.sync.dma_start(out=b, in_=t["wk"][kk * 128:(kk + 1) * 128, :])
        wk_sb.append(b)
        c = pool.tile([128, VW], F32R, tag="w", bufs=24, name=f"wv{kk}")
        nc.sync.dma_start(out=c, in_=t["wv"][kk * 128:(kk + 1) * 128, :])
        wv_sb.append(c)
    bq_sb = pool.tile([1, ECORE], F32R, tag="brow", bufs=3, name="bqs")
    nc.sync.dma_start(out=bq_sb, in_=t["bq"][:, :])
    bk_sb = pool.tile([1, ECORE], F32R, tag="brow", bufs=3, name="bks")
    nc.sync.dma_start(out=bk_sb, in_=t["bk"][:, :])
    bv_sb = pool.tile([1, VW], F32R, tag="brow", bufs=3, name="bvs")
    nc.sync.dma_start(out=bv_sb, in_=t["bv"][:, :])
    # cvec[:, h, 0] = c_h (bucket-0 zone, k > q), cvec[:, h, 1] = v31_h (far zone)
    cvec_sb = pool.tile([128, HC, 2], F32, tag="cvec", bufs=1, name="cvecs")
    nc.sync.dma_start(out=cvec_sb, in_=t["cvec"][:, :, :])
    # narrow Toeplitz band masters, one [128, BW] per head
    mastb_sb = []
    for h in range(HC):
        m = pool.tile([128, BWPAD], F32, tag="mb", bufs=HC, name=f"mb{h}")
        nc.sync.dma_start(out=m, in_=t["mast"][:, h, :])
        mastb_sb.append(m)
    ones_sb = pool.tile([1, 512], F32R, tag="ones", bufs=1, name="oness")
    nc.sync.dma_start(out=ones_sb, in_=t["ones"][:, :])

    # ---- qT / kT projections: out [256, S] as 2 tiles of [128, S] ----
    qt_sb = [pool.tile([128, S], F32R, tag="b8", bufs=13, name=f"qt{mt}") for mt in range(2)]
    kt_sb = [pool.tile([128, S], F32R, tag="b8", bufs=13, name=f"kt{mt}") for mt in range(2)]
    for dst, w_sb, b_sb in ((qt_sb, wq_sb, bq_sb), (kt_sb, wk_sb, bk_sb)):
        for mt in range(2):
            ms = slice(mt * 128, (mt + 1) * 128)
            for nt in range(4):
                ns = slice(nt * 512, (nt + 1) * 512)
                ps = psum.tile([128, 512], F32, tag="ctx", bufs=ctx_bufs, name=f"pj{mt}{nt}")
                for kk in range(8):
                    mm(ps, lhsT=rr(w_sb[kk][:, ms]), rhs=rr(ht[kk][:, ns]),
                       start=(kk == 0), stop=False)
                mm(ps, lhsT=rr(b_sb[0:1, ms]), rhs=rr(ones_sb[0:1, :]),
                   start=False, stop=True)
                evac(dst[mt][:, ns], ps)

    # ---- v projection: v[s, 260] per 128-token tile ----
    v_sb = []
    for st in range(16):
        ss = slice(st * 128, (st + 1) * 128)
        ps = psum.tile([128, VW], F32, tag="ctx", bufs=ctx_bufs, name=f"pv{st}")
        for kk in range(8):
            mm(ps, lhsT=rr(ht[kk][:, ss]), rhs=rr(wv_sb[kk]),
               start=(kk == 0), stop=False)
        mm(ps, lhsT=rr(ones_sb[0:1, 0:128]), rhs=rr(bv_sb[0:1, :]),
           start=False, stop=True)
        a = pool.tile([128, VW], F32R, tag="v", bufs=16, name=f"v{st}")
        evac(a, ps)
        v_sb.append(a)

    # ---- attention ----
    # head pairs (0,1) and (2,3); pair heads occupy PE row-groups 0-63/64-127
    # so their scores matmuls run concurrently (row-packed tile_position).
    ctxT_sb = [pool.tile([128, S], F32R, tag="b8", bufs=13, name=f"cx{p}") for p in range(2)]
    pair_outer = opts.get("pair_outer", False)
    pq_order = [(pi, qq) for pi in range(2) for qq in range(4)] if pair_outer \
        else [(pi, qq) for qq in range(4) for pi in range(2)]
    for pi, qq in pq_order:
        if True:
            qo = qq * 512
            ctx_ps = [psum.tile([65, 512], F32, tag="ctx", bufs=ctx_bufs,
                                name=f"pc{pi}{qq}{i}")
                      for i in range(2)]
            for kt in range(16):
                k0 = kt * 128
                s_ps = [psum.tile([128, 512], F32, tag="sc", bufs=sc_bufs,
                                  name=f"sc{pi}{qq}{kt}{i}")
                        for i in range(2)]
                if paired:
                    for i in range(2):
                        row0 = 64 * i
                        mm(s_ps[i],
                           lhsT=rr(kt_sb[pi][row0:row0 + 64, k0:k0 + 128]),
                           rhs=rr(qt_sb[pi][row0:row0 + 64, qo:qo + 512]),
                           start=True, stop=True)
                for i in range(2):
                    if not paired:
                        row0 = 64 * i
                        mm(s_ps[i],
                           lhsT=rr(kt_sb[pi][row0:row0 + 64, k0:k0 + 128]),
                           rhs=rr(qt_sb[pi][row0:row0 + 64, qo:qo + 512]),
                           start=True, stop=True)
                    h = 2 * pi + i
                    # band zone: q in [k0, k0+BW) -> DVE add of narrow master
                    l0 = max(0, k0 - qo)
                    l1 = min(512, k0 + BW - qo)
                    if l1 > l0:
                        nc.vector.tensor_add(
                            s_ps[i][:, l0:l1], s_ps[i][:, l0:l1],
                            mastb_sb[h][:, l0 + qo - k0:l1 + qo - k0])
                    e_sb = pool.tile([128, 512], F32R, tag="exp", bufs=exp_bufs,
                                     name=f"e{pi}{qq}{kt}{i}")
                    # exp bias: c_h for q < k0+BW (bucket-0 + band), v31_h beyond
                    lf = min(max(k0 + BW - qo, 0), 512)
                    if lf > 0:
                        nc.scalar.activation(e_sb[:, 0:lf], s_ps[i][:, 0:lf], EXP,
                                             bias=cvec_sb[:, h, 0:1], scale=1.0)
                    if lf < 512:
                        nc.scalar.activation(e_sb[:, lf:512], s_ps[i][:, lf:512], EXP,
                                             bias=cvec_sb[:, h, 1:2], scale=1.0)
                    mm(ctx_ps[i],
                       lhsT=rr(v_sb[kt][:, h * 65:(h + 1) * 65]),
                       rhs=rr(e_sb),
                       start=(kt == 0), stop=(kt == 15))
            # normalize: exact reciprocal via [1,512]->[128,4] reshape so the
            # iterative divide runs on 128 DVE lanes (DMA cannot read PSUM)
            for i in range(2):
                h = 2 * pi + i
                row0 = 64 * i
                denrow = pool.tile([1, 512], F32, tag="recip", bufs=4,
                                   name=f"dr{pi}{qq}{i}")
                nc.vector.tensor_copy(denrow, ctx_ps[i][64:65, :])
                den = pool.tile([64, 8], F32, tag="den", bufs=8, name=f"d{pi}{qq}{i}")
                nc.sync.dma_start(out=den, in_=denrow)
                rec4 = pool.tile([64, 8], F32, tag="den", bufs=8, name=f"re{pi}{qq}{i}")
                nc.vector.reciprocal(rec4, den)
                recip = pool.tile([1, 512], F32, tag="recip", bufs=4,
                                  name=f"r{pi}{qq}{i}")
                nc.sync.dma_start(out=recip, in_=rec4)
                rb = pool.tile([64, 512], F32, tag="rb", bufs=4, name=f"rb{pi}{qq}{i}")
                nc.gpsimd.partition_broadcast(rb, recip[0:1, :])
                nc.vector.tensor_mul(ctxT_sb[pi][row0:row0 + 64, qo:qo + 512],
                                     ctx_ps[i][0:64, :], rb)

    # ---- output projection: outT[e, s] partial ----
    wo_sb = []
    for ct in range(2):
        a = pool.tile([128, E], F32R, tag="w4k", bufs=2, name=f"wo{ct}")
        nc.sync.dma_start(out=a, in_=t["wo"][ct * 128:(ct + 1) * 128, :])
        wo_sb.append(a)
    for mt in range(8):
        ms = slice(mt * 128, (mt + 1) * 128)
        ot = pool.tile([128, S], F32, tag="b8", bufs=13, name=f"ot{mt}")
        for nt in range(4):
            ns = slice(nt * 512, (nt + 1) * 512)
            ps = psum.tile([128, 512], F32, tag="ctx", bufs=ctx_bufs, name=f"po{mt}{nt}")
            for ct in range(2):
                mm(ps, lhsT=rr(wo_sb[ct][:, ms]), rhs=rr(ctxT_sb[ct][:, ns]),
                   start=(ct == 0), stop=(ct == 1))
            evac(ot[:, ns], ps)
        nc.sync.dma_start(out=t["outT"][ms, :], in_=ot)


def build(iters=1, enable_asserts=False, **opts):
    """Build + compile the Bass program. Returns nc."""
    nc = bacc.Bacc("TRN2", target_bir_lowering=False, debug=False,
                   enable_asserts=enable_asserts, num_devices=NCORES)
    t = {
        "hsT": nc.dram_tensor("hsT", [E, S], F32R, kind="ExternalInput").ap(),
        "wq": nc.dram_tensor("wq", [E, ECORE], F32R, kind="ExternalInput").ap(),
        "wk": nc.dram_tensor("wk", [E, ECORE], F32R, kind="ExternalInput").ap(),
        "wv": nc.dram_tensor("wv", [E, VW], F32R, kind="ExternalInput").ap(),
        "bq": nc.dram_tensor("bq", [1, ECORE], F32R, kind="ExternalInput").ap(),
        "bk": nc.dram_tensor("bk", [1, ECORE], F32R, kind="ExternalInput").ap(),
        "bv": nc.dram_tensor("bv", [1, VW], F32R, kind="ExternalInput").ap(),
        "wo": nc.dram_tensor("wo", [ECORE, E], F32R, kind="ExternalInput").ap(),
        "ones": nc.dram_tensor("ones", [1, 512], F32R, kind="ExternalInput").ap(),
        "mast": nc.dram_tensor("mast", [128, HC, BWPAD], F32, kind="ExternalInput").ap(),
        "cvec": nc.dram_tensor("cvec", [128, HC, 2], F32, kind="ExternalInput").ap(),
        "outT": nc.dram_tensor("outT", [E, S], F32, kind="ExternalOutput").ap(),
    }
    with tile.TileContext(nc) as tc:
        with tc.tile_pool(name="sb", bufs=1) as pool, \
             tc.tile_pool(name="ps", bufs=2, space="PSUM") as psum:
            # tag capacity: b8 tiles are the big [128, 2048] f32 workhorses
            for _ in range(iters):
                _emit_body(nc, pool, psum, t, opts)
    nc.compile()
    return nc


def make_in_maps(inputs):
    """Shard full inputs into 8 per-core input maps."""
    hs = np.asarray(inputs["hidden_states"], dtype=np.float32)
    Wq = np.asarray(inputs["Wq"], dtype=np.float32)
    bq = np.asarray(inputs["bq"], dtype=np.float32)
    Wk = np.asarray(inputs["Wk"], dtype=np.float32)
    bk = np.asarray(inputs["bk"], dtype=np.float32)
    Wv = np.asarray(inputs["Wv"], dtype=np.float32)
    bv = np.asarray(inputs["bv"], dtype=np.float32)
    Wo = np.asarray(inputs["Wo"], dtype=np.float32)
    rel = np.asarray(inputs["rel_bias"], dtype=np.float32)

    # bias lookup vector per head over the band: vec_h[x] = 8*rel[bucket(x-2047), h]
    d = np.arange(-2047, 128)          # x - 2047 for x in [0, 2175)
    buck = _rel_bucket(d)              # [2175]
    # sanity: bucket saturation point matches the hardcoded band width
    assert _rel_bucket(np.array([-RP_SAT + 1]))[0] < NB - 1 <= _rel_bucket(np.array([-RP_SAT]))[0]
    idx = (np.arange(128)[:, None] - np.arange(BWPAD)[None, :]) + 2047  # [128, BWPAD]

    in_maps = []
    for core in range(NCORES):
        b, hg = core // 4, core % 4
        h0 = hg * HC
        sl = slice(h0 * HD, (h0 + HC) * HD)
        wv_aug = np.zeros((E, VW), np.float32)
        bv_aug = np.zeros((1, VW), np.float32)
        mast = np.zeros((128, HC, BWPAD), np.float32)
        cvec = np.zeros((128, HC, 2), np.float32)
        for h in range(HC):
            wv_aug[:, h * 65:h * 65 + 64] = Wv[:, (h0 + h) * HD:(h0 + h + 1) * HD]
            bv_aug[0, h * 65:h * 65 + 64] = bv[(h0 + h) * HD:(h0 + h + 1) * HD]
            bv_aug[0, h * 65 + 64] = 1.0
            vec_h = 8.0 * rel[buck, h0 + h]          # [2175]
            c_h = 8.0 * rel[0, h0 + h]               # bucket-0 constant (k > q)
            v31_h = 8.0 * rel[NB - 1, h0 + h]        # saturated far-zone constant
            mast[:, h, :] = vec_h[idx] - c_h
            cvec[:, h, 0] = c_h
            cvec[:, h, 1] = v31_h
        in_maps.append({
            "hsT": np.ascontiguousarray(hs[b].T),
            "wq": Wq[:, sl] * np.float32(SCALE),
            "wk": np.ascontiguousarray(Wk[:, sl]),
            "wv": wv_aug,
            "bq": (bq[sl] * np.float32(SCALE)).reshape(1, ECORE),
            "bk": bk[sl].reshape(1, ECORE),
            "bv": bv_aug,
            "wo": np.ascontiguousarray(Wo[sl, :]),
            "ones": np.ones((1, 512), np.float32),
            "mast": mast,
            "cvec": cvec,
        })
    return in_maps


def gather(results, inputs):
    """Sum per-core partial outT's into the full [B, S, E] output."""
    bo = np.asarray(inputs["bo"], dtype=np.float32)
    B = np.asarray(inputs["hidden_states"]).shape[0]
    out = np.zeros((B, S, E), np.float32)
    for b in range(B):
        acc = np.zeros((E, S), np.float32)
        for hg in range(4):
            acc += results[b * 4 + hg]["outT"]
        out[b] = acc.T + bo
    return out


_NC = None


def kernel(**inputs):
    global _NC
    if _NC is None:
        _NC = build()
    in_maps = make_in_maps(inputs)
    res = bass_utils.run_bass_kernel_spmd(_NC, in_maps, core_ids=list(range(NCORES)))
    return gather(res.results, inputs)
